# revision 1
# baseline (speedup 1.0000x reference)
"""Trainium2 Bass kernel for the unsupervised-entropy loss.

intra = mean_r H_r where H_r = entropy(softmax(-d2(x_r, m))).
Softmax is shift-invariant, so with unit-norm m rows the logits reduce to
z = 2 x m^T (the ||x||^2 and ||m||^2 terms drop).  Per row:
  S = sum_j exp(z_j),  W = sum_j z_j exp(z_j),  H = log S - W/S
(invariant to any constant logit shift, so no max-subtraction needed; z is
bounded by ~|2 x.m| <= ~13 which exp handles comfortably in fp32).

Device pipeline per core (x shard [32768,128]), per 1024-row block:
  1. SWDGE DMA-cast load f32->bf16, natural layout [128p, 8, 128]
  2. 8 PE transposes (bf16, vs identity) -> PSUM xT [128, 8, 128]
  3. evict xT PSUM->SBUF (split: ScalarE half / VectorE half)
  4. two bf16 matmuls accumulate z^T [128, 512] into one PSUM bank
     (lhsT = 2*m^T zero-padded to cols 0-63 / 64-127 -> chunk A on
     partitions 0-63, chunk B on partitions 64-127)
  5. ACT exp -> E bf16;  DVE z*E -> P bf16
  6. two reduce matmuls (block-indicator lhsT) -> S[2,512], W[2,512]
  7. evict stats (DVE copies S, ACT copies W) into [2, 32, 512] buffers
Final: SBUF->SBUF DMA rearranges stats to [128,256]; ACT Ln(+accum),
ACT exp(-lnS) for 1/S, DVE (W*rS -> accum). Output [128,2] per-partition
partial sums of log S and W/S; host reduces, adds the (tiny) inter term.
"""

import json

import numpy as np
import ml_dtypes

import concourse.bass as _bass
import concourse.tile as _tile
from concourse import mybir
from concourse.bass_utils import run_bass_kernel_spmd
from concourse.vector_clock import ScopedClock

F32 = mybir.dt.float32
BF16 = mybir.dt.bfloat16
N, D, K = 262144, 128, 64
NCORES = 8
NSHARD = N // NCORES          # 32768 rows per core
BLK = 1024                    # rows per block
NBLK = NSHARD // BLK          # 32 blocks
EPS = 1e-16
LAMB = 1.0


# ---- workarounds: this walrus build rejects >1 sync wait per instruction ----

def _split_multiwait(json_bytes: bytes) -> bytes:
    data = json.loads(json_bytes)
    counter = [0]
    for fn in data["functions"]:
        for blk in fn["blocks"]:
            new_insts = []
            for inst in blk["instructions"]:
                si = inst.get("sync_info")
                waits = (si or {}).get("on_wait") or []
                if len(waits) > 1:
                    for w in waits[:-1]:
                        counter[0] += 1
                        new_insts.append({
                            "debug": inst.get("debug"),
                            "engine": inst["engine"],
                            "ins": [],
                            "name": f"splitw_{counter[0]}_{inst['name']}",
                            "opcode": "EventSemaphore",
                            "outs": [],
                            "sync_info": {"on_update": [], "on_wait": [w]},
                        })
                    si["on_wait"] = [waits[-1]]
                new_insts.append(inst)
            blk["instructions"] = new_insts
    return json.dumps(data).encode()


class PatchedBass(_bass.Bass):
    def to_json_bytes(self) -> bytes:
        return _split_multiwait(super().to_json_bytes())


class SplitDrainTileContext(_tile.TileContext):
    def _drain_and_barrier(self, tick_clock, wait_clock):
        drain_inst = self.nc.sync.drain()
        wait_clock.add_sem_waits(
            drain_inst.ins, ScopedClock({None: tick_clock.global_clock})
        )
        si = drain_inst.ins.sync_info
        if si is not None and len(si.on_wait) > 1:
            waits = list(si.on_wait)
            si.on_wait = waits[:1]
            drain_inst.ins.sync_info = si
            for w in waits[1:]:
                d2 = self.nc.sync.drain()
                si2 = d2.ins.sync_info
                if si2 is None:
                    import copy
                    si2 = copy.copy(si)
                si2.on_wait = [w]
                si2.on_update = []
                d2.ins.sync_info = si2
        self.nc.all_engine_barrier()
        assert self.sems is not None
        popped = self.nc._tile_sem_poison_stack.pop()
        assert popped is self._sem_poison
        self.nc.clear_and_free_semaphores(list(self.sems.allocated().values()))
        self.nc.all_engine_barrier()


# ------------------------------ kernel build ------------------------------

_CACHE = {}


def _build():
    if "nc" in _CACHE:
        return _CACHE["nc"]
    nc = PatchedBass("TRN2", target_bir_lowering=False, debug=False)
    xs_ap = nc.dram_tensor("xs", [NSHARD, D], F32, kind="ExternalInput").ap()
    padA_ap = nc.dram_tensor("padA", [D, 128], BF16, kind="ExternalInput").ap()
    padB_ap = nc.dram_tensor("padB", [D, 128], BF16, kind="ExternalInput").ap()
    ind_ap = nc.dram_tensor("ind", [128, 2], BF16, kind="ExternalInput").ap()
    id_ap = nc.dram_tensor("ident", [128, 128], BF16, kind="ExternalInput").ap()
    out_ap = nc.dram_tensor("out", [128, 2], F32, kind="ExternalOutput").ap()

    Exp = mybir.ActivationFunctionType.Exp
    Ln = mybir.ActivationFunctionType.Ln
    MUL = mybir.AluOpType.mult
    ADD = mybir.AluOpType.add

    xs_v = xs_ap.rearrange("(b g p) d -> b p g d", p=128, g=BLK // 128)

    with SplitDrainTileContext(nc) as tc:
        with tc.tile_pool(name="const", bufs=1) as const, \
             tc.tile_pool(name="xin", bufs=3) as xin, \
             tc.tile_pool(name="xtp", bufs=3) as xtp, \
             tc.tile_pool(name="ep", bufs=3) as ep, \
             tc.tile_pool(name="stage", bufs=1) as stage, \
             tc.tile_pool(name="fin", bufs=1) as fin, \
             tc.tile_pool(name="psT", bufs=2, space="PSUM") as psTp, \
             tc.tile_pool(name="psZ", bufs=2, space="PSUM") as psZp, \
             tc.tile_pool(name="psS", bufs=2, space="PSUM") as psSp, \
             tc.tile_pool(name="psW", bufs=2, space="PSUM") as psWp:

            padA = const.tile([D, 128], BF16)
            nc.sync.dma_start(out=padA[:], in_=padA_ap[:])
            padB = const.tile([D, 128], BF16)
            nc.sync.dma_start(out=padB[:], in_=padB_ap[:])
            ind = const.tile([128, 2], BF16)
            nc.sync.dma_start(out=ind[:], in_=ind_ap[:])
            ident = const.tile([128, 128], BF16)
            nc.sync.dma_start(out=ident[:], in_=id_ap[:])

            stats_s = stage.tile([2, NBLK, 512], F32)
            stats_w = stage.tile([2, NBLK, 512], F32)

            G = BLK // 128  # 8 tiles per block
            for b in range(NBLK):
                xb = xin.tile([128, G, D], BF16)
                nc.gpsimd.dma_start(out=xb[:], in_=xs_v[b])

                psT = psTp.tile([128, G, 128], BF16)
                for g in range(G):
                    nc.tensor.transpose(psT[:, g, :], xb[:, g, :], ident[:])

                xT = xtp.tile([128, G, 128], BF16)
                # ScalarE takes 5 tiles, VectorE 3: balances ACT vs DVE busy
                # time (DVE also carries the z*E multiply and the S eviction).
                nc.scalar.copy(xT[:, 0:5, :], psT[:, 0:5, :])
                nc.vector.tensor_copy(xT[:, 5:G, :], psT[:, 5:G, :])
                xTf = xT[:].rearrange("p g r -> p (g r)")

                psZ = psZp.tile([128, 512], F32)
                nc.tensor.matmul(psZ[:], padA[:], xTf[:, 0:512],
                                 start=True, stop=False)
                nc.tensor.matmul(psZ[:], padB[:], xTf[:, 512:1024],
                                 start=False, stop=True)

                E = ep.tile([128, 512], BF16, tag="E")
                nc.scalar.activation(E[:], psZ[:], Exp)
                P = ep.tile([128, 512], BF16, tag="P")
                nc.vector.scalar_tensor_tensor(P[:], psZ[:], 1.0, E[:], MUL, MUL)

                psS = psSp.tile([2, 512], F32)
                nc.tensor.matmul(psS[:], ind[:], E[:], start=True, stop=True)
                psW = psWp.tile([2, 512], F32)
                nc.tensor.matmul(psW[:], ind[:], P[:], start=True, stop=True)

                nc.vector.tensor_copy(stats_s[:, b, :], psS[:])
                nc.scalar.copy(stats_w[:, b, :], psW[:])

            # final: rearrange per-row stats onto 128 partitions and reduce
            s128 = fin.tile([128, 256], F32)
            w128 = fin.tile([128, 256], F32)
            nc.sync.dma_start(out=s128[:],
                              in_=stats_s[:].rearrange("p a b -> p (a b)"))
            nc.sync.dma_start(out=w128[:],
                              in_=stats_w[:].rearrange("p a b -> p (a b)"))

            lnS = fin.tile([128, 256], F32)
            lsum = fin.tile([128, 1], F32)
            nc.scalar.activation(lnS[:], s128[:], Ln, accum_out=lsum[:])
            rS = fin.tile([128, 256], F32)
            nc.scalar.activation(rS[:], lnS[:], Exp, scale=-1.0)
            scr = fin.tile([128, 256], F32)
            wsum = fin.tile([128, 1], F32)
            nc.vector.scalar_tensor_tensor(scr[:], w128[:], 1.0, rS[:],
                                           MUL, MUL, accum_out=wsum[:])
            ob = fin.tile([128, 2], F32)
            nc.vector.tensor_copy(ob[:, 0:1], lsum[:])
            nc.vector.tensor_copy(ob[:, 1:2], wsum[:])
            nc.sync.dma_start(out=out_ap[:], in_=ob[:])

    _CACHE["nc"] = nc
    return nc


def _entropy_np(p):
    p = np.where(p <= 0, EPS, p)
    p = np.where(p >= 1, 1.0 - EPS, p)
    return -np.sum(p * np.log(p), axis=-1)


def kernel(x, m):
    nc = _build()

    mt2 = (2.0 * np.float64(m).T).astype(np.float32)       # [128, 64]
    padA = np.zeros((D, 128), dtype=ml_dtypes.bfloat16)
    padB = np.zeros((D, 128), dtype=ml_dtypes.bfloat16)
    padA[:, 0:K] = mt2.astype(ml_dtypes.bfloat16)
    padB[:, K:128] = mt2.astype(ml_dtypes.bfloat16)
    ind = np.zeros((128, 2), dtype=ml_dtypes.bfloat16)
    ind[0:K, 0] = 1
    ind[K:128, 1] = 1
    ident = np.eye(128, dtype=ml_dtypes.bfloat16)

    in_maps = []
    for c in range(NCORES):
        in_maps.append({
            "xs": np.ascontiguousarray(x[c * NSHARD:(c + 1) * NSHARD]),
            "padA": padA, "padB": padB, "ind": ind, "ident": ident,
        })
    _CACHE["last_in_maps"] = in_maps
    res = run_bass_kernel_spmd(nc, in_maps, core_ids=list(range(NCORES)))

    tot_ls = 0.0
    tot_ws = 0.0
    for c in range(NCORES):
        o = np.float64(res.results[c]["out"])
        tot_ls += o[:, 0].sum()
        tot_ws += o[:, 1].sum()
    intra = (tot_ls - tot_ws) / N

    # inter term on host (tiny), replicating the reference exactly
    m64 = np.float64(m)
    mu = m64.mean(axis=0)
    d2 = ((mu[None, :] - m64) ** 2).sum(axis=1)
    zl = -d2
    zl -= zl.max()
    e = np.exp(zl)
    p = e / e.sum()
    inter = _entropy_np(p)

    total = intra - LAMB * inter
    return (np.float32(total), np.float32(intra), np.float32(inter))



# revision 2
# speedup vs baseline: 1.0584x; 1.0584x over previous
"""Trainium2 Bass kernel for the unsupervised-entropy loss.

intra = mean_r H_r where H_r = entropy(softmax(-d2(x_r, m))).
Softmax is shift-invariant, so with unit-norm m rows the logits reduce to
z = 2 x m^T (the ||x||^2 and ||m||^2 terms drop).  Per row:
  S = sum_j exp(z_j),  W = sum_j z_j exp(z_j),  H = log S - W/S
(invariant to any constant logit shift, so no max-subtraction needed; z is
bounded by ~|2 x.m| <= ~13 which exp handles comfortably in fp32).

The final answer only needs sums over rows, so any row permutation is fine.
We exploit that to give the DMA large contiguous descriptors: partition p of
chunk c holds rows c*4096 + p*32 + g (g = 0..31), i.e. each partition reads
one 16 KiB contiguous run per chunk (f32->bf16 cast on load via SWDGE).

Device pipeline per core (x shard [32768,128]), per 1024-row half-block:
  1. 8 PE transposes (bf16, data stationary vs streamed identity)
     -> PSUM xT [128, 8, 128]
  2. one DVE copy evicts xT PSUM->SBUF ([128,1024] bf16, 2x perf mode)
  3. two bf16 matmuls with a shared 64-col weight (2*m^T) write
     z^T chunk A -> psZ[0:64, :], chunk B -> psZ[64:128, :] (one bank)
  4. ACT exp -> E bf16;  DVE z*E -> P bf16
  5. two reduce matmuls with zero-padded indicators accumulate into ONE
     PSUM bank psSW [4, 512] = [S_A; S_B; W_A; W_B]
  6. one ACT copy evicts psSW -> stats [4, 32, 512] f32
Final: SBUF->SBUF DMA rearranges stats to [128,256] x2; ACT Ln(+accum),
ACT exp(-lnS) for 1/S, DVE (W*rS -> accum). Output [128,2] per-partition
partial sums of log S and W/S; host reduces, adds the (tiny) inter term.
"""

import json

import numpy as np
import ml_dtypes

import concourse.bass as _bass
import concourse.tile as _tile
from concourse import mybir
from concourse.bass_utils import run_bass_kernel_spmd
from concourse.vector_clock import ScopedClock

F32 = mybir.dt.float32
BF16 = mybir.dt.bfloat16
N, D, K = 262144, 128, 64
NCORES = 8
NSHARD = N // NCORES          # 32768 rows per core
CG = 32                       # rows per partition per DMA chunk
CHUNK = 128 * CG              # 4096 rows per DMA chunk
NCHUNK = NSHARD // CHUNK      # 8 chunks
NH = 4                        # 1024-row half-blocks per chunk
NB = NCHUNK * NH              # 32 stat blocks
EPS = 1e-16
LAMB = 1.0


# ---- workarounds: this walrus build rejects >1 sync wait per instruction ----

def _split_multiwait(json_bytes: bytes) -> bytes:
    data = json.loads(json_bytes)
    counter = [0]
    for fn in data["functions"]:
        for blk in fn["blocks"]:
            new_insts = []
            for inst in blk["instructions"]:
                si = inst.get("sync_info")
                waits = (si or {}).get("on_wait") or []
                if len(waits) > 1:
                    for w in waits[:-1]:
                        counter[0] += 1
                        new_insts.append({
                            "debug": inst.get("debug"),
                            "engine": inst["engine"],
                            "ins": [],
                            "name": f"splitw_{counter[0]}_{inst['name']}",
                            "opcode": "EventSemaphore",
                            "outs": [],
                            "sync_info": {"on_update": [], "on_wait": [w]},
                        })
                    si["on_wait"] = [waits[-1]]
                new_insts.append(inst)
            blk["instructions"] = new_insts
    return json.dumps(data).encode()


class PatchedBass(_bass.Bass):
    def to_json_bytes(self) -> bytes:
        return _split_multiwait(super().to_json_bytes())


class SplitDrainTileContext(_tile.TileContext):
    def _drain_and_barrier(self, tick_clock, wait_clock):
        drain_inst = self.nc.sync.drain()
        wait_clock.add_sem_waits(
            drain_inst.ins, ScopedClock({None: tick_clock.global_clock})
        )
        si = drain_inst.ins.sync_info
        if si is not None and len(si.on_wait) > 1:
            waits = list(si.on_wait)
            si.on_wait = waits[:1]
            drain_inst.ins.sync_info = si
            for w in waits[1:]:
                d2 = self.nc.sync.drain()
                si2 = d2.ins.sync_info
                if si2 is None:
                    import copy
                    si2 = copy.copy(si)
                si2.on_wait = [w]
                si2.on_update = []
                d2.ins.sync_info = si2
        self.nc.all_engine_barrier()
        assert self.sems is not None
        popped = self.nc._tile_sem_poison_stack.pop()
        assert popped is self._sem_poison
        self.nc.clear_and_free_semaphores(list(self.sems.allocated().values()))
        self.nc.all_engine_barrier()


# ------------------------------ kernel build ------------------------------

_CACHE = {}


def _build():
    if "nc" in _CACHE:
        return _CACHE["nc"]
    nc = PatchedBass("TRN2", target_bir_lowering=False, debug=False)
    xs_ap = nc.dram_tensor("xs", [NSHARD, D], F32, kind="ExternalInput").ap()
    pad64_ap = nc.dram_tensor("pad64", [D, K], BF16, kind="ExternalInput").ap()
    indS_ap = nc.dram_tensor("indS", [128, 4], BF16, kind="ExternalInput").ap()
    indW_ap = nc.dram_tensor("indW", [128, 4], BF16, kind="ExternalInput").ap()
    id_ap = nc.dram_tensor("ident", [128, 128], BF16, kind="ExternalInput").ap()
    out_ap = nc.dram_tensor("out", [128, 2], F32, kind="ExternalOutput").ap()

    Exp = mybir.ActivationFunctionType.Exp
    Ln = mybir.ActivationFunctionType.Ln
    MUL = mybir.AluOpType.mult

    # partition p of chunk c reads rows c*4096 + p*32 + g: 16 KiB contiguous
    xs_v = xs_ap.rearrange("(c p g) d -> c p (g d)", p=128, g=CG)

    with SplitDrainTileContext(nc) as tc:
        with tc.tile_pool(name="const", bufs=1) as const, \
             tc.tile_pool(name="xin", bufs=2) as xin, \
             tc.tile_pool(name="xtp", bufs=3) as xtp, \
             tc.tile_pool(name="ep", bufs=3) as ep, \
             tc.tile_pool(name="stage", bufs=1) as stage, \
             tc.tile_pool(name="fin", bufs=1) as fin, \
             tc.tile_pool(name="psT", bufs=3, space="PSUM") as psTp, \
             tc.tile_pool(name="psZ", bufs=3, space="PSUM") as psZp, \
             tc.tile_pool(name="psSW", bufs=2, space="PSUM") as psSWp:

            pad64 = const.tile([D, K], BF16)
            nc.sync.dma_start(out=pad64[:], in_=pad64_ap[:])
            indS = const.tile([128, 4], BF16)
            nc.sync.dma_start(out=indS[:], in_=indS_ap[:])
            indW = const.tile([128, 4], BF16)
            nc.sync.dma_start(out=indW[:], in_=indW_ap[:])
            ident = const.tile([128, 128], BF16)
            nc.sync.dma_start(out=ident[:], in_=id_ap[:])

            stats = stage.tile([4, NB, 512], F32)

            for c in range(NCHUNK):
                xb = xin.tile([128, CG, D], BF16)
                nc.gpsimd.dma_start(out=xb[:], in_=xs_v[c])

                for h in range(NH):
                    b = c * NH + h
                    psT = psTp.tile([128, 8, 128], BF16)
                    for j in range(8):
                        nc.tensor.transpose(psT[:, j, :], xb[:, 8 * h + j, :],
                                            ident[:])

                    xT = xtp.tile([128, 8, 128], BF16)
                    nc.vector.tensor_copy(xT[:], psT[:])
                    xTf = xT[:].rearrange("p g r -> p (g r)")

                    psZ = psZp.tile([128, 512], F32)
                    nc.tensor.matmul(psZ[0:64, :], pad64[:], xTf[:, 0:512],
                                     start=True, stop=True)
                    nc.tensor.matmul(psZ[64:128, :], pad64[:], xTf[:, 512:1024],
                                     start=True, stop=True)

                    E = ep.tile([128, 512], BF16, tag="E")
                    nc.scalar.activation(E[:], psZ[:], Exp)
                    P = ep.tile([128, 512], BF16, tag="P")
                    nc.vector.scalar_tensor_tensor(P[:], psZ[:], 1.0, E[:],
                                                   MUL, MUL)

                    psSW = psSWp.tile([4, 512], F32)
                    nc.tensor.matmul(psSW[:], indS[:], E[:],
                                     start=True, stop=False)
                    nc.tensor.matmul(psSW[:], indW[:], P[:],
                                     start=False, stop=True)

                    nc.scalar.copy(stats[:, b, :], psSW[:])

            # final: rearrange per-row stats onto 128 partitions and reduce
            s128 = fin.tile([128, 256], F32)
            w128 = fin.tile([128, 256], F32)
            nc.sync.dma_start(out=s128[:],
                              in_=stats[0:2].rearrange("p a b -> p (a b)"))
            nc.sync.dma_start(out=w128[:],
                              in_=stats[2:4].rearrange("p a b -> p (a b)"))

            lnS = fin.tile([128, 256], F32)
            lsum = fin.tile([128, 1], F32)
            nc.scalar.activation(lnS[:], s128[:], Ln, accum_out=lsum[:])
            rS = fin.tile([128, 256], F32)
            nc.scalar.activation(rS[:], lnS[:], Exp, scale=-1.0)
            scr = fin.tile([128, 256], F32)
            wsum = fin.tile([128, 1], F32)
            nc.vector.scalar_tensor_tensor(scr[:], w128[:], 1.0, rS[:],
                                           MUL, MUL, accum_out=wsum[:])
            ob = fin.tile([128, 2], F32)
            nc.vector.tensor_copy(ob[:, 0:1], lsum[:])
            nc.vector.tensor_copy(ob[:, 1:2], wsum[:])
            nc.sync.dma_start(out=out_ap[:], in_=ob[:])

    _CACHE["nc"] = nc
    return nc


def _entropy_np(p):
    p = np.where(p <= 0, EPS, p)
    p = np.where(p >= 1, 1.0 - EPS, p)
    return -np.sum(p * np.log(p), axis=-1)


def kernel(x, m):
    nc = _build()

    mt2 = (2.0 * np.float64(m).T).astype(np.float32)       # [128, 64]
    pad64 = mt2.astype(ml_dtypes.bfloat16)
    indS = np.zeros((128, 4), dtype=ml_dtypes.bfloat16)
    indS[0:K, 0] = 1
    indS[K:128, 1] = 1
    indW = np.zeros((128, 4), dtype=ml_dtypes.bfloat16)
    indW[0:K, 2] = 1
    indW[K:128, 3] = 1
    ident = np.eye(128, dtype=ml_dtypes.bfloat16)

    in_maps = []
    for c in range(NCORES):
        in_maps.append({
            "xs": np.ascontiguousarray(x[c * NSHARD:(c + 1) * NSHARD]),
            "pad64": pad64, "indS": indS, "indW": indW, "ident": ident,
        })
    _CACHE["last_in_maps"] = in_maps
    res = run_bass_kernel_spmd(nc, in_maps, core_ids=list(range(NCORES)))

    tot_ls = 0.0
    tot_ws = 0.0
    for c in range(NCORES):
        o = np.float64(res.results[c]["out"])
        tot_ls += o[:, 0].sum()
        tot_ws += o[:, 1].sum()
    intra = (tot_ls - tot_ws) / N

    # inter term on host (tiny), replicating the reference exactly
    m64 = np.float64(m)
    mu = m64.mean(axis=0)
    d2 = ((mu[None, :] - m64) ** 2).sum(axis=1)
    zl = -d2
    zl -= zl.max()
    e = np.exp(zl)
    p = e / e.sum()
    inter = _entropy_np(p)

    total = intra - LAMB * inter
    return (np.float32(total), np.float32(intra), np.float32(inter))


# revision 5
# speedup vs baseline: 1.2924x; 1.2211x over previous
"""Trainium2 Bass kernel for the unsupervised-entropy loss.

intra = mean_r H_r where H_r = entropy(softmax(-d2(x_r, m))).
Softmax is shift-invariant, so with unit-norm m rows the logits reduce to
z = 2 x m^T (the ||x||^2 and ||m||^2 terms drop).  Per row:
  S = sum_j exp(z_j),  W = sum_j z_j exp(z_j),  H = log S - W/S
(invariant to any constant logit shift, so no max-subtraction needed; z is
bounded by ~|2 x.m| <= ~13 which exp handles comfortably in fp32).

The final answer only needs sums over rows, so any row permutation is fine.
We exploit that to give the DMA large contiguous descriptors: partition p of
chunk c holds rows c*4096 + p*32 + g (g = 0..31), i.e. each partition reads
one 16 KiB contiguous run per chunk (f32->bf16 cast on load via SWDGE).

Per 1024-row half-block i the stages are
  T(i):   8 PE transposes (bf16, data stationary)   -> psT [128, 8, 128]
  EV(i):  one DVE copy (2x perf mode)               -> xT [128, 1024] SBUF
  Z(i):   2 bf16 matmuls, shared 64-col weight 2*m^T, col-tiled so chunk A
          lands on psZ[0:64] and chunk B on psZ[64:128] (one bank)
  EXP(i): ACT exp(psZ) -> E bf16, and
  STT(i): DVE z*E -> P bf16
  R(i):   2 reduce matmuls with zero-padded indicators, col-tiled:
          S -> psSW[0:4], W -> psSW[32:36] (one bank)
  SC(i):  one ACT copy psSW[0:36] -> stats[36, 32, 512]

Engines have in-order queues, so the loop is explicitly software-pipelined:
iteration t issues T(t), EV(t-1), Z(t-2), EXP(t-3), STT(t-4), R(t-5),
SC(t-6).  Every issued op's producers completed in earlier periods, so no
engine ever head-of-line blocks on another engine's current-period work.

Final: SBUF->SBUF DMA rearranges stats to [128,256] x2; ACT Ln(+accum),
ACT exp(-lnS) for 1/S, DVE (W*rS -> accum). Output [128,2] per-partition
partial sums of log S and W/S; host reduces, adds the (tiny) inter term.
"""

import json

import numpy as np
import ml_dtypes

import concourse.bass as _bass
import concourse.tile as _tile
from concourse import mybir
from concourse.bass_utils import run_bass_kernel_spmd
from concourse.vector_clock import ScopedClock

F32 = mybir.dt.float32
BF16 = mybir.dt.bfloat16
N, D, K = 262144, 128, 64
NCORES = 8
NSHARD = N // NCORES          # 32768 rows per core
CG = 32                       # rows per partition per DMA chunk
CHUNK = 128 * CG              # 4096 rows per DMA chunk
NCHUNK = NSHARD // CHUNK      # 8 chunks
NH = 4                        # 1024-row half-blocks per chunk
NB = NCHUNK * NH              # 32 stat blocks
EPS = 1e-16
LAMB = 1.0


# ---- workarounds: this walrus build rejects >1 sync wait per instruction ----

def _split_multiwait(json_bytes: bytes) -> bytes:
    data = json.loads(json_bytes)
    counter = [0]
    for fn in data["functions"]:
        for blk in fn["blocks"]:
            new_insts = []
            for inst in blk["instructions"]:
                si = inst.get("sync_info")
                waits = (si or {}).get("on_wait") or []
                if len(waits) > 1:
                    for w in waits[:-1]:
                        counter[0] += 1
                        new_insts.append({
                            "debug": inst.get("debug"),
                            "engine": inst["engine"],
                            "ins": [],
                            "name": f"splitw_{counter[0]}_{inst['name']}",
                            "opcode": "EventSemaphore",
                            "outs": [],
                            "sync_info": {"on_update": [], "on_wait": [w]},
                        })
                    si["on_wait"] = [waits[-1]]
                new_insts.append(inst)
            blk["instructions"] = new_insts
    return json.dumps(data).encode()


class PatchedBass(_bass.Bass):
    def to_json_bytes(self) -> bytes:
        return _split_multiwait(super().to_json_bytes())


class SplitDrainTileContext(_tile.TileContext):
    def _drain_and_barrier(self, tick_clock, wait_clock):
        drain_inst = self.nc.sync.drain()
        wait_clock.add_sem_waits(
            drain_inst.ins, ScopedClock({None: tick_clock.global_clock})
        )
        si = drain_inst.ins.sync_info
        if si is not None and len(si.on_wait) > 1:
            waits = list(si.on_wait)
            si.on_wait = waits[:1]
            drain_inst.ins.sync_info = si
            for w in waits[1:]:
                d2 = self.nc.sync.drain()
                si2 = d2.ins.sync_info
                if si2 is None:
                    import copy
                    si2 = copy.copy(si)
                si2.on_wait = [w]
                si2.on_update = []
                d2.ins.sync_info = si2
        self.nc.all_engine_barrier()
        assert self.sems is not None
        popped = self.nc._tile_sem_poison_stack.pop()
        assert popped is self._sem_poison
        self.nc.clear_and_free_semaphores(list(self.sems.allocated().values()))
        self.nc.all_engine_barrier()


# ------------------------------ kernel build ------------------------------

_CACHE = {}


def _build():
    if "nc" in _CACHE:
        return _CACHE["nc"]
    nc = PatchedBass("TRN2", target_bir_lowering=False, debug=False)
    xs_ap = nc.dram_tensor("xs", [NSHARD, D], F32, kind="ExternalInput").ap()
    pad64_ap = nc.dram_tensor("pad64", [D, K], BF16, kind="ExternalInput").ap()
    indS_ap = nc.dram_tensor("indS", [128, 4], BF16, kind="ExternalInput").ap()
    indW_ap = nc.dram_tensor("indW", [128, 4], BF16, kind="ExternalInput").ap()
    id_ap = nc.dram_tensor("ident", [128, 128], BF16, kind="ExternalInput").ap()
    out_ap = nc.dram_tensor("out", [128, 2], F32, kind="ExternalOutput").ap()

    Exp = mybir.ActivationFunctionType.Exp
    Ln = mybir.ActivationFunctionType.Ln
    MUL = mybir.AluOpType.mult

    # partition p of chunk c reads rows c*4096 + p*32 + g: 16 KiB contiguous
    xs_v = xs_ap.rearrange("(c p g) d -> c p (g d)", p=128, g=CG)

    with SplitDrainTileContext(nc) as tc:
        with tc.tile_pool(name="const", bufs=1) as const, \
             tc.tile_pool(name="xin", bufs=3) as xin, \
             tc.tile_pool(name="xtp", bufs=3) as xtp, \
             tc.tile_pool(name="ep", bufs=3) as ep, \
             tc.tile_pool(name="stage", bufs=1) as stage, \
             tc.tile_pool(name="fin", bufs=1) as fin, \
             tc.tile_pool(name="psT", bufs=2, space="PSUM") as psTp, \
             tc.tile_pool(name="psZ", bufs=3, space="PSUM") as psZp, \
             tc.tile_pool(name="psSW", bufs=2, space="PSUM") as psSWp:

            pad64 = const.tile([D, K], BF16)
            nc.sync.dma_start(out=pad64[:], in_=pad64_ap[:])
            indS = const.tile([128, 4], BF16)
            nc.sync.dma_start(out=indS[:], in_=indS_ap[:])
            indW = const.tile([128, 4], BF16)
            nc.sync.dma_start(out=indW[:], in_=indW_ap[:])
            ident = const.tile([128, 128], BF16)
            nc.sync.dma_start(out=ident[:], in_=id_ap[:])

            stats = stage.tile([36, NB, 512], F32)

            xbs = {}          # chunk -> xin tile
            psTs = {}         # block -> psT tile
            xTs = {}          # block -> xT SBUF tile
            psZs = {}         # block -> psZ tile
            Es = {}
            Ps = {}
            psSWs = {}

            def load_chunk(c):
                xbs[c] = xin.tile([128, CG, D], BF16, name="xb", tag="xb")
                nc.gpsimd.dma_start(out=xbs[c][:], in_=xs_v[c])

            load_chunk(0)
            load_chunk(1)

            for t in range(NB + 6):
                if t % NH == 0 and t < NB:
                    c2 = t // NH + 2
                    if c2 < NCHUNK:
                        load_chunk(c2)

                # T(t): transposes on PE
                if t < NB:
                    xb = xbs[t // NH]
                    psTs[t] = psTp.tile([128, 8, 128], BF16, name="psT", tag="psT")
                    for j in range(8):
                        nc.tensor.transpose(psTs[t][:, j, :],
                                            xb[:, (t % NH) * 8 + j, :],
                                            ident[:])

                # EV(t-1): evict xT on DVE
                i = t - 1
                if 0 <= i < NB:
                    xTs[i] = xtp.tile([128, 8, 128], BF16, name="xT", tag="xT")
                    nc.vector.tensor_copy(xTs[i][:], psTs[i][:])
                    del psTs[i]

                # Z(t-2): logits matmuls on PE (col-tiled halves)
                i = t - 2
                if 0 <= i < NB:
                    xTf = xTs[i][:].rearrange("p g r -> p (g r)")
                    psZs[i] = psZp.tile([128, 512], F32, name="psZ", tag="psZ")
                    nc.tensor.matmul(psZs[i][0:64, :], pad64[:],
                                     xTf[:, 0:512], start=True, stop=True,
                                     tile_position=(0, 0))
                    nc.tensor.matmul(psZs[i][64:128, :], pad64[:],
                                     xTf[:, 512:1024], start=True, stop=True,
                                     tile_position=(0, 64))
                    del xTs[i]

                # EXP(t-3) on ACT
                i = t - 3
                if 0 <= i < NB:
                    Es[i] = ep.tile([128, 512], BF16, tag="E", name="E")
                    nc.scalar.activation(Es[i][:], psZs[i][:], Exp)

                # STT(t-4) on DVE
                i = t - 4
                if 0 <= i < NB:
                    Ps[i] = ep.tile([128, 512], BF16, tag="P", name="P")
                    nc.vector.scalar_tensor_tensor(Ps[i][:], psZs[i][:], 1.0,
                                                   Es[i][:], MUL, MUL)
                    del psZs[i]

                # R(t-5): reduce matmuls on PE (col-tiled S and W)
                i = t - 5
                if 0 <= i < NB:
                    psSWs[i] = psSWp.tile([36, 512], F32, name="psSW", tag="psSW")
                    nc.tensor.matmul(psSWs[i][0:4, :], indS[:], Es[i][:],
                                     start=True, stop=True,
                                     tile_position=(0, 0))
                    nc.tensor.matmul(psSWs[i][32:36, :], indW[:], Ps[i][:],
                                     start=True, stop=True,
                                     tile_position=(0, 32))
                    del Es[i]
                    del Ps[i]

                # SC(t-6): stats eviction on ACT
                i = t - 6
                if 0 <= i < NB:
                    nc.scalar.copy(stats[:, i, :], psSWs[i][:])
                    del psSWs[i]

            # final: rearrange per-row stats onto 128 partitions and reduce
            s128 = fin.tile([128, 256], F32)
            w128 = fin.tile([128, 256], F32)
            nc.sync.dma_start(out=s128[:],
                              in_=stats[0:2].rearrange("p a b -> p (a b)"))
            nc.sync.dma_start(out=w128[:],
                              in_=stats[32:34].rearrange("p a b -> p (a b)"))

            lnS = fin.tile([128, 256], F32)
            lsum = fin.tile([128, 1], F32)
            nc.scalar.activation(lnS[:], s128[:], Ln, accum_out=lsum[:])
            rS = fin.tile([128, 256], F32)
            nc.scalar.activation(rS[:], lnS[:], Exp, scale=-1.0)
            scr = fin.tile([128, 256], F32)
            wsum = fin.tile([128, 1], F32)
            nc.vector.scalar_tensor_tensor(scr[:], w128[:], 1.0, rS[:],
                                           MUL, MUL, accum_out=wsum[:])
            ob = fin.tile([128, 2], F32)
            nc.vector.tensor_copy(ob[:, 0:1], lsum[:])
            nc.vector.tensor_copy(ob[:, 1:2], wsum[:])
            nc.sync.dma_start(out=out_ap[:], in_=ob[:])

    _CACHE["nc"] = nc
    return nc


def _entropy_np(p):
    p = np.where(p <= 0, EPS, p)
    p = np.where(p >= 1, 1.0 - EPS, p)
    return -np.sum(p * np.log(p), axis=-1)


def kernel(x, m):
    nc = _build()

    mt2 = (2.0 * np.float64(m).T).astype(np.float32)       # [128, 64]
    pad64 = mt2.astype(ml_dtypes.bfloat16)
    indS = np.zeros((128, 4), dtype=ml_dtypes.bfloat16)
    indS[0:K, 0] = 1
    indS[K:128, 1] = 1
    indW = np.zeros((128, 4), dtype=ml_dtypes.bfloat16)
    indW[0:K, 0] = 1
    indW[K:128, 1] = 1
    ident = np.eye(128, dtype=ml_dtypes.bfloat16)

    in_maps = []
    for c in range(NCORES):
        in_maps.append({
            "xs": np.ascontiguousarray(x[c * NSHARD:(c + 1) * NSHARD]),
            "pad64": pad64, "indS": indS, "indW": indW, "ident": ident,
        })
    _CACHE["last_in_maps"] = in_maps
    res = run_bass_kernel_spmd(nc, in_maps, core_ids=list(range(NCORES)))

    tot_ls = 0.0
    tot_ws = 0.0
    for c in range(NCORES):
        o = np.float64(res.results[c]["out"])
        tot_ls += o[:, 0].sum()
        tot_ws += o[:, 1].sum()
    intra = (tot_ls - tot_ws) / N

    # inter term on host (tiny), replicating the reference exactly
    m64 = np.float64(m)
    mu = m64.mean(axis=0)
    d2 = ((mu[None, :] - m64) ** 2).sum(axis=1)
    zl = -d2
    zl -= zl.max()
    e = np.exp(zl)
    p = e / e.sum()
    inter = _entropy_np(p)

    total = intra - LAMB * inter
    return (np.float32(total), np.float32(intra), np.float32(inter))


# revision 8
# speedup vs baseline: 1.3651x; 1.0562x over previous
"""Trainium2 Bass kernel for the unsupervised-entropy loss.

intra = mean_r H_r where H_r = entropy(softmax(-d2(x_r, m))).
Softmax is shift-invariant, so with unit-norm m rows the logits reduce to
z = 2 x m^T (the ||x||^2 and ||m||^2 terms drop).  Per row:
  S = sum_j exp(z_j),  W = sum_j z_j exp(z_j),  H = log S - W/S
(invariant to any constant logit shift, so no max-subtraction needed; z is
bounded by ~|2 x.m| <= ~13 which exp handles comfortably in fp32).

The final answer only needs sums over rows, so any row permutation is fine.
We exploit that to give the DMA large contiguous descriptors: partition p of
chunk c holds rows base_c + p*CG + g (g = 0..CG-1), i.e. each partition reads
one contiguous run per chunk (f32->bf16 cast on load via SWDGE).  Chunks are
2 MiB except the last two (1 MiB) so the compute trailing the final DMA is
short.  All constants ship in ONE DMA so the 8 DMA-completion semaphore
lanes are never oversubscribed at startup (a lane collision there made the
first transpose wait on the *third* chunk's DMA).

Per 1024-row half-block i the stages are
  T(i):   8 PE transposes (bf16, data stationary)   -> psT [128, 8, 128]
  EV(i):  one DVE copy (2x perf mode)               -> xT [128, 1024] SBUF
  Z(i):   2 bf16 matmuls, shared 64-col weight 2*m^T, col-tiled: chunk A
          -> psZ[0:64], chunk B -> psZ[64:128] (one bank)
  EXP(i): ACT exp(psZ) -> E bf16
  STT(i): DVE z*E -> P bf16
  R(i):   2 reduce matmuls, zero-padded indicators, col-tiled:
          S -> psSW[0:4], W -> psSW[32:36] (one bank)
  SC(i):  one ACT copy psSW[0:36] -> stats[36, 32, 512]

Engines have in-order queues, so the loop is explicitly software-pipelined:
iteration t issues T(t), EV(t-1), Z(t-2), EXP(t-3), STT(t-4), R(t-5),
SC(t-6) — every issued op's producers completed in earlier periods.  A
burst of dummy ident matmuls runs during the first chunk's DMA window to
flip the PE's HAM clock gate to 2.4 GHz before real work arrives.

Final reduction is split in halves; the first half (stats blocks 0:16) is
folded into the pipeline shadow around iterations 24-30.  Host reduces the
[128,2] per-partition partial sums and adds the (tiny) inter term.
"""

import json

import numpy as np
import ml_dtypes

import concourse.bass as _bass
import concourse.tile as _tile
from concourse import mybir
from concourse.bass_utils import run_bass_kernel_spmd
from concourse.vector_clock import ScopedClock

F32 = mybir.dt.float32
BF16 = mybir.dt.bfloat16
N, D, K = 262144, 128, 64
NCORES = 8
NSHARD = N // NCORES          # 32768 rows per core
NBIG = 7                      # 2 MiB chunks (CG=32)
NSMALL = 2                    # 1 MiB chunks (CG=16) at the end
BIGROWS = 128 * 32
SMALLROWS = 128 * 16
NB = NBIG * 4 + NSMALL * 2    # 32 half-blocks of 1024 rows
EPS = 1e-16
LAMB = 1.0
NWARM = 44                    # HAM warm-up matmuls


# ---- workarounds: this walrus build rejects >1 sync wait per instruction ----

def _split_multiwait(json_bytes: bytes) -> bytes:
    data = json.loads(json_bytes)
    counter = [0]
    for fn in data["functions"]:
        for blk in fn["blocks"]:
            new_insts = []
            for inst in blk["instructions"]:
                si = inst.get("sync_info")
                waits = (si or {}).get("on_wait") or []
                if len(waits) > 1:
                    for w in waits[:-1]:
                        counter[0] += 1
                        new_insts.append({
                            "debug": inst.get("debug"),
                            "engine": inst["engine"],
                            "ins": [],
                            "name": f"splitw_{counter[0]}_{inst['name']}",
                            "opcode": "EventSemaphore",
                            "outs": [],
                            "sync_info": {"on_update": [], "on_wait": [w]},
                        })
                    si["on_wait"] = [waits[-1]]
                new_insts.append(inst)
            blk["instructions"] = new_insts
    return json.dumps(data).encode()


class PatchedBass(_bass.Bass):
    def to_json_bytes(self) -> bytes:
        return _split_multiwait(super().to_json_bytes())


class SplitDrainTileContext(_tile.TileContext):
    def _drain_and_barrier(self, tick_clock, wait_clock):
        drain_inst = self.nc.sync.drain()
        wait_clock.add_sem_waits(
            drain_inst.ins, ScopedClock({None: tick_clock.global_clock})
        )
        si = drain_inst.ins.sync_info
        if si is not None and len(si.on_wait) > 1:
            waits = list(si.on_wait)
            si.on_wait = waits[:1]
            drain_inst.ins.sync_info = si
            for w in waits[1:]:
                d2 = self.nc.sync.drain()
                si2 = d2.ins.sync_info
                if si2 is None:
                    import copy
                    si2 = copy.copy(si)
                si2.on_wait = [w]
                si2.on_update = []
                d2.ins.sync_info = si2
        self.nc.all_engine_barrier()
        assert self.sems is not None
        popped = self.nc._tile_sem_poison_stack.pop()
        assert popped is self._sem_poison
        self.nc.clear_and_free_semaphores(list(self.sems.allocated().values()))
        self.nc.all_engine_barrier()


# ------------------------------ kernel build ------------------------------

_CACHE = {}


def _build():
    if "nc" in _CACHE:
        return _CACHE["nc"]
    nc = PatchedBass("TRN2", target_bir_lowering=False, debug=False)
    xs_ap = nc.dram_tensor("xs", [NSHARD, D], F32, kind="ExternalInput").ap()
    consts_ap = nc.dram_tensor("consts", [128, 200], BF16,
                               kind="ExternalInput").ap()
    out_ap = nc.dram_tensor("out", [128, 2], F32, kind="ExternalOutput").ap()

    Exp = mybir.ActivationFunctionType.Exp
    Ln = mybir.ActivationFunctionType.Ln
    MUL = mybir.AluOpType.mult
    ADD = mybir.AluOpType.add

    bigrows = NBIG * BIGROWS
    xs_big = xs_ap[0:bigrows].rearrange("(c p g) d -> c p (g d)", p=128, g=32)
    xs_small = xs_ap[bigrows:NSHARD].rearrange("(c p g) d -> c p (g d)",
                                               p=128, g=16)

    # half-block index -> (chunk, half, chunk_key)
    def locate(b):
        if b < NBIG * 4:
            return b // 4, b % 4, ("b", b // 4)
        bb = b - NBIG * 4
        return bb // 2, bb % 2, ("s", bb // 2)

    with SplitDrainTileContext(nc) as tc:
        with tc.tile_pool(name="const", bufs=1) as const, \
             tc.tile_pool(name="xin", bufs=3) as xin, \
             tc.tile_pool(name="xtp", bufs=3) as xtp, \
             tc.tile_pool(name="ep", bufs=3) as ep, \
             tc.tile_pool(name="stage", bufs=1) as stage, \
             tc.tile_pool(name="fin", bufs=1) as fin, \
             tc.tile_pool(name="psT", bufs=2, space="PSUM") as psTp, \
             tc.tile_pool(name="psZ", bufs=3, space="PSUM") as psZp, \
             tc.tile_pool(name="psSW", bufs=2, space="PSUM") as psSWp:

            consts = const.tile([128, 200], BF16)
            nc.sync.dma_start(out=consts[:], in_=consts_ap[:])
            pad64 = consts[:, 0:64]
            indS = consts[:, 64:68]
            indW = consts[:, 68:72]
            ident = consts[:, 72:200]

            stats = stage.tile([36, NB, 512], F32)

            xbs = {}
            psTs = {}
            xTs = {}
            psZs = {}
            Es = {}
            Ps = {}
            psSWs = {}

            def load_chunk(key):
                kind, c = key
                if kind == "b":
                    xbs[key] = xin.tile([128, 32, D], BF16, name="xb",
                                        tag="xb")
                    nc.gpsimd.dma_start(out=xbs[key][:], in_=xs_big[c])
                else:
                    xbs[key] = xin.tile([128, 16, D], BF16, name="xbs",
                                        tag="xbs", bufs=2)
                    nc.gpsimd.dma_start(out=xbs[key][:], in_=xs_small[c])

            load_chunk(("b", 0))
            load_chunk(("b", 1))

            # HAM warm-up: dummy matmuls while the first chunk loads
            warm = psSWp.tile([128, 128], F32, name="warm", tag="warm",
                              bufs=1)
            for _ in range(NWARM):
                nc.tensor.matmul(warm[:], ident, ident, start=True, stop=True)

            # final-stage tiles (first half folded into the loop)
            s128a = fin.tile([128, 128], F32)
            w128a = fin.tile([128, 128], F32)
            lnA = fin.tile([128, 128], F32)
            lsA = fin.tile([128, 1], F32)
            rSA = fin.tile([128, 128], F32)
            scrA = fin.tile([128, 128], F32)
            wsA = fin.tile([128, 1], F32)

            for t in range(NB + 6):
                # chunk prefetch: big chunks 2 ahead, then the small tail
                if t % 4 == 0 and t < NB:
                    c2 = t // 4 + 2
                    if c2 < NBIG:
                        load_chunk(("b", c2))
                    elif c2 == NBIG:
                        load_chunk(("s", 0))
                    elif c2 == NBIG + 1:
                        load_chunk(("s", 1))

                # T(t): transposes on PE
                if t < NB:
                    c, h, key = locate(t)
                    xb = xbs[key]
                    psTs[t] = psTp.tile([128, 8, 128], BF16, name="psT",
                                        tag="psT")
                    for j in range(8):
                        nc.tensor.transpose(psTs[t][:, j, :],
                                            xb[:, h * 8 + j, :], ident)

                # EV(t-1): evict xT on DVE
                i = t - 1
                if 0 <= i < NB:
                    xTs[i] = xtp.tile([128, 8, 128], BF16, name="xT",
                                      tag="xT")
                    nc.vector.tensor_copy(xTs[i][:], psTs[i][:])
                    del psTs[i]

                # Z(t-2): logits matmuls on PE (col-tiled halves)
                i = t - 2
                if 0 <= i < NB:
                    xTf = xTs[i][:].rearrange("p g r -> p (g r)")
                    psZs[i] = psZp.tile([128, 512], F32, name="psZ",
                                        tag="psZ")
                    nc.tensor.matmul(psZs[i][0:64, :], pad64,
                                     xTf[:, 0:512], start=True, stop=True,
                                     tile_position=(0, 0))
                    nc.tensor.matmul(psZs[i][64:128, :], pad64,
                                     xTf[:, 512:1024], start=True, stop=True,
                                     tile_position=(0, 64))
                    del xTs[i]

                # EXP(t-3) on ACT
                i = t - 3
                if 0 <= i < NB:
                    Es[i] = ep.tile([128, 512], BF16, tag="E", name="E")
                    nc.scalar.activation(Es[i][:], psZs[i][:], Exp)

                # STT(t-4) on DVE
                i = t - 4
                if 0 <= i < NB:
                    Ps[i] = ep.tile([128, 512], BF16, tag="P", name="P")
                    nc.vector.scalar_tensor_tensor(Ps[i][:], psZs[i][:], 1.0,
                                                   Es[i][:], MUL, MUL)
                    del psZs[i]

                # R(t-5): reduce matmuls on PE (col-tiled S and W)
                i = t - 5
                if 0 <= i < NB:
                    psSWs[i] = psSWp.tile([36, 512], F32, name="psSW",
                                          tag="psSW")
                    nc.tensor.matmul(psSWs[i][0:4, :], indS, Es[i][:],
                                     start=True, stop=True,
                                     tile_position=(0, 0))
                    nc.tensor.matmul(psSWs[i][32:36, :], indW, Ps[i][:],
                                     start=True, stop=True,
                                     tile_position=(0, 32))
                    del Es[i]
                    del Ps[i]

                # SC(t-6): stats eviction on ACT
                i = t - 6
                if 0 <= i < NB:
                    nc.scalar.copy(stats[:, i, :], psSWs[i][:])
                    del psSWs[i]

                # first-half final reduction, spread across iterations
                if t == 24:   # SC(15) issued at t=21, done well before
                    nc.sync.dma_start(
                        out=s128a[:],
                        in_=stats[0:2, 0:16].rearrange("p a b -> p (a b)"))
                    nc.sync.dma_start(
                        out=w128a[:],
                        in_=stats[32:34, 0:16].rearrange("p a b -> p (a b)"))
                elif t == 27:
                    nc.scalar.activation(lnA[:], s128a[:], Ln,
                                         accum_out=lsA[:])
                elif t == 28:
                    nc.scalar.activation(rSA[:], lnA[:], Exp, scale=-1.0)
                elif t == 30:
                    nc.vector.scalar_tensor_tensor(scrA[:], w128a[:], 1.0,
                                                   rSA[:], MUL, MUL,
                                                   accum_out=wsA[:])

            # second-half final reduction
            s128b = fin.tile([128, 128], F32)
            w128b = fin.tile([128, 128], F32)
            nc.sync.dma_start(
                out=s128b[:],
                in_=stats[0:2, 16:32].rearrange("p a b -> p (a b)"))
            nc.sync.dma_start(
                out=w128b[:],
                in_=stats[32:34, 16:32].rearrange("p a b -> p (a b)"))

            lnB = fin.tile([128, 128], F32)
            lsB = fin.tile([128, 1], F32)
            nc.scalar.activation(lnB[:], s128b[:], Ln, accum_out=lsB[:])
            rSB = fin.tile([128, 128], F32)
            nc.scalar.activation(rSB[:], lnB[:], Exp, scale=-1.0)
            scrB = fin.tile([128, 128], F32)
            wsB = fin.tile([128, 1], F32)
            nc.vector.scalar_tensor_tensor(scrB[:], w128b[:], 1.0, rSB[:],
                                           MUL, MUL, accum_out=wsB[:])
            ob = fin.tile([128, 2], F32)
            nc.vector.tensor_tensor(ob[:, 0:1], lsA[:], lsB[:], ADD)
            nc.vector.tensor_tensor(ob[:, 1:2], wsA[:], wsB[:], ADD)
            nc.sync.dma_start(out=out_ap[:], in_=ob[:])

    _CACHE["nc"] = nc
    return nc


def _entropy_np(p):
    p = np.where(p <= 0, EPS, p)
    p = np.where(p >= 1, 1.0 - EPS, p)
    return -np.sum(p * np.log(p), axis=-1)


def kernel(x, m):
    nc = _build()

    mt2 = (2.0 * np.float64(m).T).astype(np.float32)       # [128, 64]
    consts = np.zeros((128, 200), dtype=ml_dtypes.bfloat16)
    consts[:, 0:64] = mt2.astype(ml_dtypes.bfloat16)       # pad64
    consts[0:K, 64] = 1                                    # indS col 0
    consts[K:128, 65] = 1                                  # indS col 1
    consts[0:K, 68] = 1                                    # indW col 0
    consts[K:128, 69] = 1                                  # indW col 1
    consts[:, 72:200] = np.eye(128, dtype=ml_dtypes.bfloat16)

    in_maps = []
    for c in range(NCORES):
        in_maps.append({
            "xs": np.ascontiguousarray(x[c * NSHARD:(c + 1) * NSHARD]),
            "consts": consts,
        })
    _CACHE["last_in_maps"] = in_maps
    res = run_bass_kernel_spmd(nc, in_maps, core_ids=list(range(NCORES)))

    tot_ls = 0.0
    tot_ws = 0.0
    for c in range(NCORES):
        o = np.float64(res.results[c]["out"])
        tot_ls += o[:, 0].sum()
        tot_ws += o[:, 1].sum()
    intra = (tot_ls - tot_ws) / N

    # inter term on host (tiny), replicating the reference exactly
    m64 = np.float64(m)
    mu = m64.mean(axis=0)
    d2 = ((mu[None, :] - m64) ** 2).sum(axis=1)
    zl = -d2
    zl -= zl.max()
    e = np.exp(zl)
    p = e / e.sum()
    inter = _entropy_np(p)

    total = intra - LAMB * inter
    return (np.float32(total), np.float32(intra), np.float32(inter))


# revision 11
# speedup vs baseline: 1.3992x; 1.0250x over previous
"""Trainium2 Bass kernel for the unsupervised-entropy loss.

intra = mean_r H_r where H_r = entropy(softmax(-d2(x_r, m))).
Softmax is shift-invariant, so with unit-norm m rows the logits reduce to
z = 2 x m^T (the ||x||^2 and ||m||^2 terms drop).  Per row:
  S = sum_j exp(z_j),  W = sum_j z_j exp(z_j),  H = log S - W/S
(invariant to any constant logit shift, so no max-subtraction needed; z is
bounded by ~|2 x.m| <= ~13 which exp handles comfortably in fp32).

The final answer only needs sums over rows, so any row permutation is fine.
We exploit that to give the DMA large contiguous descriptors: partition p of
chunk c holds rows base_c + p*CG + g (g = 0..CG-1), i.e. each partition reads
one contiguous run per chunk (f32->bf16 cast on load via SWDGE).  Chunks are
2 MiB except the last two (1 MiB) so the compute trailing the final DMA is
short.  All constants ship in ONE DMA.  The x chunk loads bypass the tile
dependency tracker entirely: they land in a raw 3-slot SBUF ring and are
synchronized with two explicit semaphores (xSem: DMA completions, +16 per
chunk; warSem: +1 per xT eviction, gating slot reuse).  Tile's automatic
DMA wait assignment is conservative -- consumers ended up waiting on the
*latest* issued chunk, keeping the pipeline ~2 chunks behind the DMA
stream; explicit thresholds make every transpose wait on exactly its own
chunk.

Per 1024-row half-block i the stages are
  T(i):   8 PE transposes (bf16, data stationary)   -> psT [128, 8, 128]
  EV(i):  one DVE copy (2x perf mode)               -> xT [128, 1024] SBUF
  Z(i):   2 bf16 matmuls, shared 64-col weight 2*m^T, col-tiled: chunk A
          -> psZ[0:64], chunk B -> psZ[64:128] (one bank)
  EXP(i): ACT exp(psZ) -> E bf16
  STT(i): DVE z*E -> P bf16
  R(i):   2 reduce matmuls, zero-padded indicators, col-tiled:
          S -> psSW[0:4], W -> psSW[32:36] (one bank)
  SC(i):  one ACT copy psSW[0:36] -> stats[36, 32, 512]

Engines have in-order queues, so the loop is explicitly software-pipelined:
iteration t issues T(t), EV(t-1), Z(t-2), EXP(t-3), STT(t-4), R(t-5),
SC(t-6) — every issued op's producers completed in earlier periods.  A
burst of dummy ident matmuls runs during the first chunk's DMA window to
flip the PE's HAM clock gate to 2.4 GHz before real work arrives.

Final reduction is split in halves; the first half (stats blocks 0:16) is
folded into the pipeline shadow around iterations 24-30.  Host reduces the
[128,2] per-partition partial sums and adds the (tiny) inter term.
"""

import json

import numpy as np
import ml_dtypes

import concourse.bass as _bass
import concourse.tile as _tile
from concourse import mybir
from concourse.bass_utils import run_bass_kernel_spmd
from concourse.vector_clock import ScopedClock

F32 = mybir.dt.float32
BF16 = mybir.dt.bfloat16
N, D, K = 262144, 128, 64
NCORES = 8
NSHARD = N // NCORES          # 32768 rows per core
NBIG = 7                      # 2 MiB chunks (CG=32)
NSMALL = 2                    # 1 MiB chunks (CG=16) at the end
BIGROWS = 128 * 32
SMALLROWS = 128 * 16
NB = NBIG * 4 + NSMALL * 2    # 32 half-blocks of 1024 rows
EPS = 1e-16
LAMB = 1.0
NWARM = 44                    # HAM warm-up matmuls


# ---- workarounds: this walrus build rejects >1 sync wait per instruction ----

def _split_multiwait(json_bytes: bytes) -> bytes:
    data = json.loads(json_bytes)
    counter = [0]
    for fn in data["functions"]:
        for blk in fn["blocks"]:
            new_insts = []
            for inst in blk["instructions"]:
                si = inst.get("sync_info")
                waits = (si or {}).get("on_wait") or []
                if len(waits) > 1:
                    for w in waits[:-1]:
                        counter[0] += 1
                        new_insts.append({
                            "debug": inst.get("debug"),
                            "engine": inst["engine"],
                            "ins": [],
                            "name": f"splitw_{counter[0]}_{inst['name']}",
                            "opcode": "EventSemaphore",
                            "outs": [],
                            "sync_info": {"on_update": [], "on_wait": [w]},
                        })
                    si["on_wait"] = [waits[-1]]
                new_insts.append(inst)
            blk["instructions"] = new_insts
    return json.dumps(data).encode()


class PatchedBass(_bass.Bass):
    def to_json_bytes(self) -> bytes:
        return _split_multiwait(super().to_json_bytes())


class SplitDrainTileContext(_tile.TileContext):
    def _drain_and_barrier(self, tick_clock, wait_clock):
        drain_inst = self.nc.sync.drain()
        wait_clock.add_sem_waits(
            drain_inst.ins, ScopedClock({None: tick_clock.global_clock})
        )
        si = drain_inst.ins.sync_info
        if si is not None and len(si.on_wait) > 1:
            waits = list(si.on_wait)
            si.on_wait = waits[:1]
            drain_inst.ins.sync_info = si
            for w in waits[1:]:
                d2 = self.nc.sync.drain()
                si2 = d2.ins.sync_info
                if si2 is None:
                    import copy
                    si2 = copy.copy(si)
                si2.on_wait = [w]
                si2.on_update = []
                d2.ins.sync_info = si2
        self.nc.all_engine_barrier()
        assert self.sems is not None
        popped = self.nc._tile_sem_poison_stack.pop()
        assert popped is self._sem_poison
        self.nc.clear_and_free_semaphores(list(self.sems.allocated().values()))
        self.nc.all_engine_barrier()


# ------------------------------ kernel build ------------------------------

_CACHE = {}


def _build():
    if "nc" in _CACHE:
        return _CACHE["nc"]
    nc = PatchedBass("TRN2", target_bir_lowering=False, debug=False)
    xs_ap = nc.dram_tensor("xs", [NSHARD, D], F32, kind="ExternalInput").ap()
    consts_ap = nc.dram_tensor("consts", [128, 200], BF16,
                               kind="ExternalInput").ap()
    out_ap = nc.dram_tensor("out", [128, 2], F32, kind="ExternalOutput").ap()

    Exp = mybir.ActivationFunctionType.Exp
    Ln = mybir.ActivationFunctionType.Ln
    MUL = mybir.AluOpType.mult
    ADD = mybir.AluOpType.add

    bigrows = NBIG * BIGROWS
    xs_big = xs_ap[0:bigrows].rearrange("(c p g) d -> c p (g d)", p=128, g=32)
    xs_small = xs_ap[bigrows:NSHARD].rearrange("(c p g) d -> c p (g d)",
                                               p=128, g=16)

    NCHUNK = NBIG + NSMALL
    iters_of = [4] * NBIG + [2] * NSMALL
    cum_iters = [0]
    for n_ in iters_of:
        cum_iters.append(cum_iters[-1] + n_)

    # half-block index -> (chunk index, half within chunk)
    def locate(b):
        if b < NBIG * 4:
            return b // 4, b % 4
        bb = b - NBIG * 4
        return NBIG + bb // 2, bb % 2

    from contextlib import ExitStack
    octx = ExitStack()
    # one raw SBUF buffer per chunk: tile's per-tensor dependency tracking
    # then gives every transpose exactly one DMA dep (its own chunk)
    xbuf = []
    for j in range(NBIG + NSMALL):
        gcols = 32 * D if j < NBIG else 16 * D
        xbuf.append(octx.enter_context(
            nc.sbuf_tensor(f"xbuf{j}", [128, gcols], BF16)))

    with SplitDrainTileContext(nc) as tc:
        with tc.tile_pool(name="const", bufs=1) as const, \
             tc.tile_pool(name="xtp", bufs=3) as xtp, \
             tc.tile_pool(name="ep", bufs=3) as ep, \
             tc.tile_pool(name="stage", bufs=1) as stage, \
             tc.tile_pool(name="fin", bufs=1) as fin, \
             tc.tile_pool(name="psT", bufs=2, space="PSUM") as psTp, \
             tc.tile_pool(name="psZ", bufs=3, space="PSUM") as psZp, \
             tc.tile_pool(name="psSW", bufs=2, space="PSUM") as psSWp:

            consts = const.tile([128, 200], BF16)
            nc.sync.dma_start(out=consts[:], in_=consts_ap[:])
            pad64 = consts[:, 0:64]
            indS = consts[:, 64:68]
            indW = consts[:, 68:72]
            ident = consts[:, 72:200]

            stats = stage.tile([36, NB, 512], F32)

            # issue ALL chunk loads up front; buffers are exclusive per
            # chunk so the SWDGE queue streams them back to back
            for j in range(NCHUNK):
                slot = xbuf[j].ap()
                if j < NBIG:
                    nc.gpsimd.dma_start(out=slot[:], in_=xs_big[j])
                else:
                    nc.gpsimd.dma_start(out=slot[:], in_=xs_small[j - NBIG])

            # HAM warm-up: dummy matmuls while the first chunk loads
            warm = psSWp.tile([128, 128], F32, name="warm", tag="warm",
                              bufs=1)
            for _ in range(NWARM):
                nc.tensor.matmul(warm[:], ident, ident, start=True, stop=True)

            # final-stage tiles (first half folded into the loop)
            s128a = fin.tile([128, 128], F32)
            w128a = fin.tile([128, 128], F32)
            lnA = fin.tile([128, 128], F32)
            lsA = fin.tile([128, 1], F32)
            rSA = fin.tile([128, 128], F32)
            scrA = fin.tile([128, 128], F32)
            wsA = fin.tile([128, 1], F32)

            psTs = {}
            xTs = {}
            psZs = {}
            Es = {}
            Ps = {}
            psSWs = {}

            for t in range(NB + 6):
                # T(t): transposes on PE, reading the raw ring
                if t < NB:
                    cj, h = locate(t)
                    slot = xbuf[cj].ap()
                    psTs[t] = psTp.tile([128, 8, 128], BF16, name="psT",
                                        tag="psT")
                    for j in range(8):
                        nc.tensor.transpose(
                            psTs[t][:, j, :],
                            slot[:, (h * 8 + j) * 128:(h * 8 + j + 1) * 128],
                            ident)

                # EV(t-1): evict xT on DVE; +1 on warSem frees ring slots
                i = t - 1
                if 0 <= i < NB:
                    xTs[i] = xtp.tile([128, 8, 128], BF16, name="xT",
                                      tag="xT")
                    nc.vector.tensor_copy(xTs[i][:], psTs[i][:])
                    del psTs[i]

                # Z(t-2): logits matmuls on PE (col-tiled halves)
                i = t - 2
                if 0 <= i < NB:
                    xTf = xTs[i][:].rearrange("p g r -> p (g r)")
                    psZs[i] = psZp.tile([128, 512], F32, name="psZ",
                                        tag="psZ")
                    nc.tensor.matmul(psZs[i][0:64, :], pad64,
                                     xTf[:, 0:512], start=True, stop=True,
                                     tile_position=(0, 0))
                    nc.tensor.matmul(psZs[i][64:128, :], pad64,
                                     xTf[:, 512:1024], start=True, stop=True,
                                     tile_position=(0, 64))
                    del xTs[i]

                # EXP(t-3) on ACT
                i = t - 3
                if 0 <= i < NB:
                    Es[i] = ep.tile([128, 512], BF16, tag="E", name="E")
                    nc.scalar.activation(Es[i][:], psZs[i][:], Exp)

                # STT(t-4) on DVE
                i = t - 4
                if 0 <= i < NB:
                    Ps[i] = ep.tile([128, 512], BF16, tag="P", name="P")
                    nc.vector.scalar_tensor_tensor(Ps[i][:], psZs[i][:], 1.0,
                                                   Es[i][:], MUL, MUL)
                    del psZs[i]

                # R(t-5): reduce matmuls on PE (col-tiled S and W)
                i = t - 5
                if 0 <= i < NB:
                    psSWs[i] = psSWp.tile([36, 512], F32, name="psSW",
                                          tag="psSW")
                    nc.tensor.matmul(psSWs[i][0:4, :], indS, Es[i][:],
                                     start=True, stop=True,
                                     tile_position=(0, 0))
                    nc.tensor.matmul(psSWs[i][32:36, :], indW, Ps[i][:],
                                     start=True, stop=True,
                                     tile_position=(0, 32))
                    del Es[i]
                    del Ps[i]

                # SC(t-6): stats eviction on ACT
                i = t - 6
                if 0 <= i < NB:
                    nc.scalar.copy(stats[:, i, :], psSWs[i][:])
                    del psSWs[i]

                # first-half final reduction, spread across iterations
                if t == 24:   # SC(15) issued at t=21, done well before
                    nc.sync.dma_start(
                        out=s128a[:],
                        in_=stats[0:2, 0:16].rearrange("p a b -> p (a b)"))
                    nc.sync.dma_start(
                        out=w128a[:],
                        in_=stats[32:34, 0:16].rearrange("p a b -> p (a b)"))
                elif t == 27:
                    nc.scalar.activation(lnA[:], s128a[:], Ln,
                                         accum_out=lsA[:])
                elif t == 28:
                    nc.scalar.activation(rSA[:], lnA[:], Exp, scale=-1.0)
                elif t == 30:
                    nc.vector.scalar_tensor_tensor(scrA[:], w128a[:], 1.0,
                                                   rSA[:], MUL, MUL,
                                                   accum_out=wsA[:])

            # second-half final reduction
            s128b = fin.tile([128, 128], F32)
            w128b = fin.tile([128, 128], F32)
            nc.sync.dma_start(
                out=s128b[:],
                in_=stats[0:2, 16:32].rearrange("p a b -> p (a b)"))
            nc.sync.dma_start(
                out=w128b[:],
                in_=stats[32:34, 16:32].rearrange("p a b -> p (a b)"))

            lnB = fin.tile([128, 128], F32)
            lsB = fin.tile([128, 1], F32)
            nc.scalar.activation(lnB[:], s128b[:], Ln, accum_out=lsB[:])
            rSB = fin.tile([128, 128], F32)
            nc.scalar.activation(rSB[:], lnB[:], Exp, scale=-1.0)
            scrB = fin.tile([128, 128], F32)
            wsB = fin.tile([128, 1], F32)
            nc.vector.scalar_tensor_tensor(scrB[:], w128b[:], 1.0, rSB[:],
                                           MUL, MUL, accum_out=wsB[:])
            ob = fin.tile([128, 2], F32)
            nc.vector.tensor_tensor(ob[:, 0:1], lsA[:], lsB[:], ADD)
            nc.vector.tensor_tensor(ob[:, 1:2], wsA[:], wsB[:], ADD)
            nc.sync.dma_start(out=out_ap[:], in_=ob[:])

    _CACHE["nc"] = nc
    _CACHE["octx"] = octx
    return nc


def _entropy_np(p):
    p = np.where(p <= 0, EPS, p)
    p = np.where(p >= 1, 1.0 - EPS, p)
    return -np.sum(p * np.log(p), axis=-1)


def kernel(x, m):
    nc = _build()

    mt2 = (2.0 * np.float64(m).T).astype(np.float32)       # [128, 64]
    consts = np.zeros((128, 200), dtype=ml_dtypes.bfloat16)
    consts[:, 0:64] = mt2.astype(ml_dtypes.bfloat16)       # pad64
    consts[0:K, 64] = 1                                    # indS col 0
    consts[K:128, 65] = 1                                  # indS col 1
    consts[0:K, 68] = 1                                    # indW col 0
    consts[K:128, 69] = 1                                  # indW col 1
    consts[:, 72:200] = np.eye(128, dtype=ml_dtypes.bfloat16)

    in_maps = []
    for c in range(NCORES):
        in_maps.append({
            "xs": np.ascontiguousarray(x[c * NSHARD:(c + 1) * NSHARD]),
            "consts": consts,
        })
    _CACHE["last_in_maps"] = in_maps
    res = run_bass_kernel_spmd(nc, in_maps, core_ids=list(range(NCORES)))

    tot_ls = 0.0
    tot_ws = 0.0
    for c in range(NCORES):
        o = np.float64(res.results[c]["out"])
        tot_ls += o[:, 0].sum()
        tot_ws += o[:, 1].sum()
    intra = (tot_ls - tot_ws) / N

    # inter term on host (tiny), replicating the reference exactly
    m64 = np.float64(m)
    mu = m64.mean(axis=0)
    d2 = ((mu[None, :] - m64) ** 2).sum(axis=1)
    zl = -d2
    zl -= zl.max()
    e = np.exp(zl)
    p = e / e.sum()
    inter = _entropy_np(p)

    total = intra - LAMB * inter
    return (np.float32(total), np.float32(intra), np.float32(inter))


# revision 12
# speedup vs baseline: 1.4646x; 1.0467x over previous
"""Trainium2 Bass kernel for the unsupervised-entropy loss.

intra = mean_r H_r where H_r = entropy(softmax(-d2(x_r, m))).
Softmax is shift-invariant, so with unit-norm m rows the logits reduce to
z = 2 x m^T (the ||x||^2 and ||m||^2 terms drop).  Per row:
  S = sum_j exp(z_j),  W = sum_j z_j exp(z_j),  H = log S - W/S
(invariant to any constant logit shift, so no max-subtraction needed; z is
bounded by ~|2 x.m| <= ~13 which exp handles comfortably in fp32).

The final answer only needs sums over rows, so any row permutation is fine.
We exploit that to give the DMA large contiguous descriptors: partition p of
chunk c holds rows base_c + p*CG + g (g = 0..CG-1), i.e. each partition reads
one contiguous run per chunk (f32->bf16 cast on load via SWDGE).  Chunks are
2 MiB except the last two (1 MiB) so the compute trailing the final DMA is
short.  All constants ship in ONE DMA.  The x chunk loads bypass the tile
dependency tracker entirely: they land in a raw 3-slot SBUF ring and are
synchronized with two explicit semaphores (xSem: DMA completions, +16 per
chunk; warSem: +1 per xT eviction, gating slot reuse).  Tile's automatic
DMA wait assignment is conservative -- consumers ended up waiting on the
*latest* issued chunk, keeping the pipeline ~2 chunks behind the DMA
stream; explicit thresholds make every transpose wait on exactly its own
chunk.

Per 1024-row half-block i the stages are
  T(i):   8 PE transposes (bf16, data stationary)   -> psT [128, 8, 128]
  EV(i):  one DVE copy (2x perf mode)               -> xT [128, 1024] SBUF
  Z(i):   2 bf16 matmuls, shared 64-col weight 2*m^T, col-tiled: chunk A
          -> psZ[0:64], chunk B -> psZ[64:128] (one bank)
  EXP(i): ACT exp(psZ) -> E bf16
  STT(i): DVE z*E -> P bf16
  R(i):   2 reduce matmuls, zero-padded indicators, col-tiled:
          S -> psSW[0:4], W -> psSW[32:36] (one bank)
  SC(i):  one ACT copy psSW[0:36] -> stats[36, 32, 512]

Engines have in-order queues, so the loop is explicitly software-pipelined:
iteration t issues T(t), EV(t-1), Z(t-2), EXP(t-3), STT(t-4), R(t-5),
SC(t-6) — every issued op's producers completed in earlier periods.  A
burst of dummy ident matmuls runs during the first chunk's DMA window to
flip the PE's HAM clock gate to 2.4 GHz before real work arrives.

Final reduction is split in halves; the first half (stats blocks 0:16) is
folded into the pipeline shadow around iterations 24-30.  Host reduces the
[128,2] per-partition partial sums and adds the (tiny) inter term.
"""

import json

import numpy as np
import ml_dtypes

import concourse.bass as _bass
import concourse.tile as _tile
from concourse import mybir
from concourse.bass_utils import run_bass_kernel_spmd
from concourse.vector_clock import ScopedClock

F32 = mybir.dt.float32
BF16 = mybir.dt.bfloat16
N, D, K = 262144, 128, 64
NCORES = 8
NSHARD = N // NCORES          # 32768 rows per core
NBIG = 7                      # 2 MiB chunks (CG=32)
NSMALL = 2                    # 1 MiB chunks (CG=16) at the end
BIGROWS = 128 * 32
SMALLROWS = 128 * 16
NB = NBIG * 4 + NSMALL * 2    # 32 half-blocks of 1024 rows
EPS = 1e-16
LAMB = 1.0
NWARM = 44                    # HAM warm-up matmuls


# ---- workarounds: this walrus build rejects >1 sync wait per instruction ----

def _split_multiwait(json_bytes: bytes) -> bytes:
    data = json.loads(json_bytes)
    counter = [0]
    for fn in data["functions"]:
        for blk in fn["blocks"]:
            new_insts = []
            for inst in blk["instructions"]:
                si = inst.get("sync_info")
                waits = (si or {}).get("on_wait") or []
                if len(waits) > 1:
                    for w in waits[:-1]:
                        counter[0] += 1
                        new_insts.append({
                            "debug": inst.get("debug"),
                            "engine": inst["engine"],
                            "ins": [],
                            "name": f"splitw_{counter[0]}_{inst['name']}",
                            "opcode": "EventSemaphore",
                            "outs": [],
                            "sync_info": {"on_update": [], "on_wait": [w]},
                        })
                    si["on_wait"] = [waits[-1]]
                new_insts.append(inst)
            blk["instructions"] = new_insts
    return json.dumps(data).encode()


class PatchedBass(_bass.Bass):
    def to_json_bytes(self) -> bytes:
        return _split_multiwait(super().to_json_bytes())


class SplitDrainTileContext(_tile.TileContext):
    def _drain_and_barrier(self, tick_clock, wait_clock):
        drain_inst = self.nc.sync.drain()
        wait_clock.add_sem_waits(
            drain_inst.ins, ScopedClock({None: tick_clock.global_clock})
        )
        si = drain_inst.ins.sync_info
        if si is not None and len(si.on_wait) > 1:
            waits = list(si.on_wait)
            si.on_wait = waits[:1]
            drain_inst.ins.sync_info = si
            for w in waits[1:]:
                d2 = self.nc.sync.drain()
                si2 = d2.ins.sync_info
                if si2 is None:
                    import copy
                    si2 = copy.copy(si)
                si2.on_wait = [w]
                si2.on_update = []
                d2.ins.sync_info = si2
        self.nc.all_engine_barrier()
        assert self.sems is not None
        popped = self.nc._tile_sem_poison_stack.pop()
        assert popped is self._sem_poison
        self.nc.clear_and_free_semaphores(list(self.sems.allocated().values()))
        self.nc.all_engine_barrier()


# ------------------------------ kernel build ------------------------------

_CACHE = {}


def _build():
    if "nc" in _CACHE:
        return _CACHE["nc"]
    nc = PatchedBass("TRN2", target_bir_lowering=False, debug=False)
    xs_ap = nc.dram_tensor("xs", [NSHARD, D], F32, kind="ExternalInput").ap()
    consts_ap = nc.dram_tensor("consts", [128, 200], BF16,
                               kind="ExternalInput").ap()
    out_ap = nc.dram_tensor("out", [128, 2], F32, kind="ExternalOutput").ap()

    Exp = mybir.ActivationFunctionType.Exp
    Ln = mybir.ActivationFunctionType.Ln
    MUL = mybir.AluOpType.mult
    ADD = mybir.AluOpType.add

    bigrows = NBIG * BIGROWS
    xs_big = xs_ap[0:bigrows].rearrange("(c p g) d -> c p (g d)", p=128, g=32)
    xs_small = xs_ap[bigrows:NSHARD].rearrange("(c p g) d -> c p (g d)",
                                               p=128, g=16)

    NCHUNK = NBIG + NSMALL
    iters_of = [4] * NBIG + [2] * NSMALL
    cum_iters = [0]
    for n_ in iters_of:
        cum_iters.append(cum_iters[-1] + n_)

    # half-block index -> (chunk index, half within chunk)
    def locate(b):
        if b < NBIG * 4:
            return b // 4, b % 4
        bb = b - NBIG * 4
        return NBIG + bb // 2, bb % 2

    from contextlib import ExitStack
    octx = ExitStack()
    # one raw SBUF buffer per chunk: tile's per-tensor dependency tracking
    # then gives every transpose exactly one DMA dep (its own chunk)
    xbuf = []
    for j in range(NBIG + NSMALL):
        gcols = 32 * D if j < NBIG else 16 * D
        xbuf.append(octx.enter_context(
            nc.sbuf_tensor(f"xbuf{j}", [128, gcols], BF16)))

    with SplitDrainTileContext(nc) as tc:
        with tc.tile_pool(name="const", bufs=1) as const, \
             tc.tile_pool(name="xtp", bufs=3) as xtp, \
             tc.tile_pool(name="ep", bufs=3) as ep, \
             tc.tile_pool(name="stage", bufs=1) as stage, \
             tc.tile_pool(name="fin", bufs=1) as fin, \
             tc.tile_pool(name="psT", bufs=2, space="PSUM") as psTp, \
             tc.tile_pool(name="psZ", bufs=3, space="PSUM") as psZp, \
             tc.tile_pool(name="psSW", bufs=2, space="PSUM") as psSWp:

            consts = const.tile([128, 200], BF16)
            nc.sync.dma_start(out=consts[:], in_=consts_ap[:])
            pad64 = consts[:, 0:64]
            indS = consts[:, 64:68]
            indW = consts[:, 68:72]
            ident = consts[:, 72:200]

            stats = stage.tile([36, NB, 512], F32)

            # issue ALL chunk loads up front; buffers are exclusive per
            # chunk so the SWDGE queue streams them back to back
            for j in range(NCHUNK):
                slot = xbuf[j].ap()
                if j < NBIG:
                    nc.gpsimd.dma_start(out=slot[:], in_=xs_big[j])
                else:
                    nc.gpsimd.dma_start(out=slot[:], in_=xs_small[j - NBIG])

            # HAM warm-up: dummy matmuls while the first chunk loads
            warm = psSWp.tile([128, 128], F32, name="warm", tag="warm",
                              bufs=1)
            for _ in range(NWARM):
                nc.tensor.matmul(warm[:], ident, ident, start=True, stop=True)

            # final-stage tiles (ranges A=[0:16], C=[16:24] folded into
            # the loop; D=[24:32] runs in the epilogue)
            s128a = fin.tile([128, 128], F32)
            w128a = fin.tile([128, 128], F32)
            lnA = fin.tile([128, 128], F32)
            lsA = fin.tile([128, 1], F32)
            rSA = fin.tile([128, 128], F32)
            scrA = fin.tile([128, 128], F32)
            wsA = fin.tile([128, 1], F32)
            s128c = fin.tile([128, 64], F32)
            w128c = fin.tile([128, 64], F32)
            lnC = fin.tile([128, 64], F32)
            lsC = fin.tile([128, 1], F32)
            rSC = fin.tile([128, 64], F32)
            scrC = fin.tile([128, 64], F32)
            wsC = fin.tile([128, 1], F32)

            psTs = {}
            xTs = {}
            psZs = {}
            Es = {}
            Ps = {}
            psSWs = {}

            for t in range(NB + 6):
                # T(t): transposes on PE, reading the raw ring
                if t < NB:
                    cj, h = locate(t)
                    slot = xbuf[cj].ap()
                    psTs[t] = psTp.tile([128, 8, 128], BF16, name="psT",
                                        tag="psT")
                    for j in range(8):
                        nc.tensor.transpose(
                            psTs[t][:, j, :],
                            slot[:, (h * 8 + j) * 128:(h * 8 + j + 1) * 128],
                            ident)

                # EV(t-1): evict xT on DVE; +1 on warSem frees ring slots
                i = t - 1
                if 0 <= i < NB:
                    xTs[i] = xtp.tile([128, 8, 128], BF16, name="xT",
                                      tag="xT")
                    nc.vector.tensor_copy(xTs[i][:], psTs[i][:])
                    del psTs[i]

                # Z(t-2): logits matmuls on PE (col-tiled halves)
                i = t - 2
                if 0 <= i < NB:
                    xTf = xTs[i][:].rearrange("p g r -> p (g r)")
                    psZs[i] = psZp.tile([128, 512], F32, name="psZ",
                                        tag="psZ")
                    nc.tensor.matmul(psZs[i][0:64, :], pad64,
                                     xTf[:, 0:512], start=True, stop=True,
                                     tile_position=(0, 0))
                    nc.tensor.matmul(psZs[i][64:128, :], pad64,
                                     xTf[:, 512:1024], start=True, stop=True,
                                     tile_position=(0, 64))
                    del xTs[i]

                # EXP(t-3) on ACT
                i = t - 3
                if 0 <= i < NB:
                    Es[i] = ep.tile([128, 512], BF16, tag="E", name="E")
                    nc.scalar.activation(Es[i][:], psZs[i][:], Exp)

                # STT(t-4) on DVE
                i = t - 4
                if 0 <= i < NB:
                    Ps[i] = ep.tile([128, 512], BF16, tag="P", name="P")
                    nc.vector.scalar_tensor_tensor(Ps[i][:], psZs[i][:], 1.0,
                                                   Es[i][:], MUL, MUL)
                    del psZs[i]

                # R(t-5): reduce matmuls on PE (col-tiled S and W)
                i = t - 5
                if 0 <= i < NB:
                    psSWs[i] = psSWp.tile([36, 512], F32, name="psSW",
                                          tag="psSW")
                    nc.tensor.matmul(psSWs[i][0:4, :], indS, Es[i][:],
                                     start=True, stop=True,
                                     tile_position=(0, 0))
                    nc.tensor.matmul(psSWs[i][32:36, :], indW, Ps[i][:],
                                     start=True, stop=True,
                                     tile_position=(0, 32))
                    del Es[i]
                    del Ps[i]

                # SC(t-6): stats eviction on ACT
                i = t - 6
                if 0 <= i < NB:
                    nc.scalar.copy(stats[:, i, :], psSWs[i][:])
                    del psSWs[i]

                # in-loop final reduction for ranges A and C, spread out
                # so no engine head-of-line blocks on a DMA completion
                if t == 22:   # SC(15) issued at t=21
                    nc.sync.dma_start(
                        out=s128a[:],
                        in_=stats[0:2, 0:16].rearrange("p a b -> p (a b)"))
                    nc.sync.dma_start(
                        out=w128a[:],
                        in_=stats[32:34, 0:16].rearrange("p a b -> p (a b)"))
                elif t == 28:
                    nc.scalar.activation(lnA[:], s128a[:], Ln,
                                         accum_out=lsA[:])
                elif t == 30:
                    nc.scalar.activation(rSA[:], lnA[:], Exp, scale=-1.0)
                elif t == 32:
                    nc.vector.scalar_tensor_tensor(scrA[:], w128a[:], 1.0,
                                                   rSA[:], MUL, MUL,
                                                   accum_out=wsA[:])
                    nc.sync.dma_start(
                        out=s128c[:],
                        in_=stats[0:2, 16:24].rearrange("p a b -> p (a b)"))
                    nc.sync.dma_start(
                        out=w128c[:],
                        in_=stats[32:34, 16:24].rearrange("p a b -> p (a b)"))
                elif t == 36:
                    nc.scalar.activation(lnC[:], s128c[:], Ln,
                                         accum_out=lsC[:])
                elif t == 37:
                    nc.scalar.activation(rSC[:], lnC[:], Exp, scale=-1.0)

            # epilogue: finish range C, then range D = blocks [24:32]
            nc.vector.scalar_tensor_tensor(scrC[:], w128c[:], 1.0, rSC[:],
                                           MUL, MUL, accum_out=wsC[:])
            s128d = fin.tile([128, 64], F32)
            w128d = fin.tile([128, 64], F32)
            nc.sync.dma_start(
                out=s128d[:],
                in_=stats[0:2, 24:32].rearrange("p a b -> p (a b)"))
            nc.sync.dma_start(
                out=w128d[:],
                in_=stats[32:34, 24:32].rearrange("p a b -> p (a b)"))

            lnD = fin.tile([128, 64], F32)
            lsD = fin.tile([128, 1], F32)
            nc.scalar.activation(lnD[:], s128d[:], Ln, accum_out=lsD[:])
            rSD = fin.tile([128, 64], F32)
            nc.scalar.activation(rSD[:], lnD[:], Exp, scale=-1.0)
            scrD = fin.tile([128, 64], F32)
            wsD = fin.tile([128, 1], F32)
            nc.vector.scalar_tensor_tensor(scrD[:], w128d[:], 1.0, rSD[:],
                                           MUL, MUL, accum_out=wsD[:])
            lsAC = fin.tile([128, 1], F32)
            wsAC = fin.tile([128, 1], F32)
            nc.vector.tensor_tensor(lsAC[:], lsA[:], lsC[:], ADD)
            nc.vector.tensor_tensor(wsAC[:], wsA[:], wsC[:], ADD)
            ob = fin.tile([128, 2], F32)
            nc.vector.tensor_tensor(ob[:, 0:1], lsAC[:], lsD[:], ADD)
            nc.vector.tensor_tensor(ob[:, 1:2], wsAC[:], wsD[:], ADD)
            nc.sync.dma_start(out=out_ap[:], in_=ob[:])

    _CACHE["nc"] = nc
    _CACHE["octx"] = octx
    return nc


def _entropy_np(p):
    p = np.where(p <= 0, EPS, p)
    p = np.where(p >= 1, 1.0 - EPS, p)
    return -np.sum(p * np.log(p), axis=-1)


def kernel(x, m):
    nc = _build()

    mt2 = (2.0 * np.float64(m).T).astype(np.float32)       # [128, 64]
    consts = np.zeros((128, 200), dtype=ml_dtypes.bfloat16)
    consts[:, 0:64] = mt2.astype(ml_dtypes.bfloat16)       # pad64
    consts[0:K, 64] = 1                                    # indS col 0
    consts[K:128, 65] = 1                                  # indS col 1
    consts[0:K, 68] = 1                                    # indW col 0
    consts[K:128, 69] = 1                                  # indW col 1
    consts[:, 72:200] = np.eye(128, dtype=ml_dtypes.bfloat16)

    in_maps = []
    for c in range(NCORES):
        in_maps.append({
            "xs": np.ascontiguousarray(x[c * NSHARD:(c + 1) * NSHARD]),
            "consts": consts,
        })
    _CACHE["last_in_maps"] = in_maps
    res = run_bass_kernel_spmd(nc, in_maps, core_ids=list(range(NCORES)))

    tot_ls = 0.0
    tot_ws = 0.0
    for c in range(NCORES):
        o = np.float64(res.results[c]["out"])
        tot_ls += o[:, 0].sum()
        tot_ws += o[:, 1].sum()
    intra = (tot_ls - tot_ws) / N

    # inter term on host (tiny), replicating the reference exactly
    m64 = np.float64(m)
    mu = m64.mean(axis=0)
    d2 = ((mu[None, :] - m64) ** 2).sum(axis=1)
    zl = -d2
    zl -= zl.max()
    e = np.exp(zl)
    p = e / e.sum()
    inter = _entropy_np(p)

    total = intra - LAMB * inter
    return (np.float32(total), np.float32(intra), np.float32(inter))
